# revision 6
# baseline (speedup 1.0000x reference)
"""CharacterLevelCNN on 8 Trainium2 NeuronCores.

Sharding: data-parallel convs (8 samples/core), then output-channel-sharded
FC1, then oc-sharded FC2 + contraction-partial FC3. Three SPMD launches with
host gather/concat/sum (unshard) between them.

Conv matmuls run in float32r (~1.7e-4 rel err), FC matmuls in exact fp32.
"""
import numpy as np
from contextlib import ExitStack

import concourse.bass as bass
import concourse.tile as tile
import concourse.mybir as mybir
from concourse import bacc
from concourse.bass_utils import run_bass_kernel_spmd
from concourse.masks import make_identity

F32 = mybir.dt.float32
F32R = mybir.dt.float32r
AX = mybir.AxisListType
OP = mybir.AluOpType
ACT = mybir.ActivationFunctionType

B, L, C, E = 64, 1014, 70, 32
NCORES = 8
BS = B // NCORES            # 8 samples per core
ROWS = BS * L               # 8112
L1, P1 = 1008, 336          # conv1 valid out / after pool3
L2, P2 = 330, 110           # conv2
L3, L4, L5 = 108, 106, 104  # conv3..5 (no pool)
L6, P6 = 102, 34            # conv6 + pool3
NG = 8 * P6                 # 272 feature groups of 128 channels
FLAT = 1024 * P6            # 34816
FC = 2048
NCLS = 14
OCS = FC // NCORES          # 256 fc out-channels per core

CORES = list(range(NCORES))


def _build_conv_nc():
    nc = bacc.Bacc("TRN2", target_bir_lowering=False, debug=False)
    xin = nc.dram_tensor("xin", [ROWS, C], F32, kind="ExternalInput").ap()
    embin = nc.dram_tensor("embin", [C, E], F32, kind="ExternalInput").ap()
    w1t = nc.dram_tensor("w1t", [224, 1024], F32, kind="ExternalInput").ap()
    wps = {}
    for i, ntap in ((2, 7), (3, 3), (4, 3), (5, 3), (6, 3)):
        wps[i] = nc.dram_tensor(f"w{i}p", [1024, ntap, 1024], F32,
                                kind="ExternalInput").ap()
    bs = {}
    for i in range(1, 7):
        bs[i] = nc.dram_tensor(f"b{i}", [1024], F32, kind="ExternalInput").ap()
    feat = nc.dram_tensor("feat", [NG, 128, BS], F32, kind="ExternalOutput").ap()

    with tile.TileContext(nc) as tc, ExitStack() as ctx:
        const = ctx.enter_context(tc.tile_pool(name="const", bufs=1))
        epool = ctx.enter_context(tc.tile_pool(name="epool", bufs=3))
        big = ctx.enter_context(tc.tile_pool(name="big", bufs=1))
        xcp = ctx.enter_context(tc.tile_pool(name="xcp", bufs=2))
        wpool = ctx.enter_context(tc.tile_pool(name="wpool", bufs=4))
        vpool = ctx.enter_context(tc.tile_pool(name="vpool", bufs=4))
        ppool = ctx.enter_context(tc.tile_pool(name="ppool", bufs=8, space="PSUM"))

        ident = const.tile([128, 128], F32, tag="ident")
        make_identity(nc, ident[:])
        iota = const.tile([128, C], F32, tag="iota")
        nc.gpsimd.iota(iota[:], pattern=[[-1, C]], base=C, channel_multiplier=0,
                       allow_small_or_imprecise_dtypes=True)
        emb_sb = const.tile([C, E], F32, tag="emb")
        nc.sync.dma_start(emb_sb[:], embin)
        w1a = const.tile([128, 1024], F32R, tag="w1a")
        nc.sync.dma_start(w1a[:], w1t[:128].bitcast(F32R))
        w1b = const.tile([96, 1024], F32R, tag="w1b")
        nc.sync.dma_start(w1b[:], w1t[128:224].bitcast(F32R))
        bsb = {}
        for i in range(1, 7):
            bsb[i] = const.tile([128, 8], F32, tag=f"bsb{i}", name=f"bsb{i}")
            nc.sync.dma_start(bsb[i][:], bs[i].rearrange("(m p) -> p m", p=128))

        # big activation buffers; disjoint-lifetime pairs share a tag
        x1 = big.tile([128, 8, BS, P1], F32R, tag="x1")
        x2 = big.tile([128, 8, BS, P2], F32R, tag="x2_x4")
        # x3/x4/x5/x6 allocated later (tag reuse needs alloc order = use order)

        # ---- per-sample: embed -> im2col -> conv1 ----
        TT1 = 504
        for s in range(BS):
            eT = epool.tile([E, L], F32, tag="eT")
            nt = (L + 127) // 128
            for i in range(nt):
                r0 = i * 128
                nr = min(128, L - r0)
                xt = epool.tile([128, C], F32, tag="xt")
                nc.sync.dma_start(xt[:nr], xin[s * L + r0: s * L + r0 + nr, :])
                mx = epool.tile([128, 1], F32, tag="mx")
                nc.vector.tensor_reduce(mx[:nr], xt[:nr], axis=AX.X, op=OP.max)
                ge = epool.tile([128, C], F32, tag="ge")
                nc.vector.tensor_scalar(ge[:nr], xt[:nr], mx[:nr], None, OP.is_ge)
                nc.vector.tensor_tensor(ge[:nr], ge[:nr], iota[:nr], OP.mult)
                mx2 = epool.tile([128, 1], F32, tag="mx2")
                nc.vector.tensor_reduce(mx2[:nr], ge[:nr], axis=AX.X, op=OP.max)
                nc.vector.tensor_scalar(ge[:nr], ge[:nr], mx2[:nr], None, OP.is_ge)
                pt = ppool.tile([C, 128], F32, tag="ps")
                nc.tensor.transpose(pt[:, :nr], ge[:nr], ident[:nr, :nr])
                mts = epool.tile([C, 128], F32, tag="mts")
                nc.vector.tensor_copy(mts[:, :nr], pt[:, :nr])
                pe = ppool.tile([E, 128], F32, tag="ps")
                nc.tensor.matmul(pe[:, :nr], emb_sb[:], mts[:, :nr],
                                 start=True, stop=True)
                nc.vector.tensor_copy(eT[:, r0:r0 + nr], pe[:, :nr])
            # im2col: 7 shifted copies (taps 0-3 -> xc0, taps 4-6 -> xc1)
            xc0 = xcp.tile([128, L1], F32R, tag="xc0")
            xc1 = xcp.tile([96, L1], F32R, tag="xc1")
            for k in range(7):
                dst, kk = (xc0, k) if k < 4 else (xc1, k - 4)
                nc.sync.dma_start(dst[32 * kk:32 * kk + 32, :],
                                  eT[:, k:k + L1].bitcast(F32R))
            for m in range(8):
                for T in range(2):
                    ps = ppool.tile([128, TT1], F32, tag="ps")
                    nc.tensor.matmul(ps[:], w1a[:, m * 128:(m + 1) * 128],
                                     xc0[:, T * TT1:(T + 1) * TT1],
                                     start=True, stop=False)
                    nc.tensor.matmul(ps[:], w1b[:, m * 128:(m + 1) * 128],
                                     xc1[:, T * TT1:(T + 1) * TT1],
                                     start=False, stop=True)
                    tmp = vpool.tile([128, TT1 // 3], F32, tag="pool1")
                    nc.vector.tensor_reduce(
                        tmp[:], ps[:].rearrange("p (t three) -> p t three", three=3),
                        axis=AX.X, op=OP.max)
                    nc.scalar.activation(x1[:, m, s, T * 168:(T + 1) * 168], tmp[:],
                                         ACT.Relu, bias=bsb[1][:, m:m + 1])

        # ---- conv2: K = 1024x7, 8 psum banks = 8 samples ----
        for m in range(8):
            pss = [ppool.tile([128, L2], F32, tag="ps", name=f"ps2_{m}_{si}") for si in range(BS)]
            ki = 0
            for kc in range(8):
                for tap in range(7):
                    wt = wpool.tile([128, 128], F32R, tag="w2t")
                    nc.sync.dma_start(
                        wt[:], wps[2][kc * 128:(kc + 1) * 128, tap,
                                      m * 128:(m + 1) * 128].bitcast(F32R))
                    for s in range(BS):
                        nc.tensor.matmul(pss[s][:], wt[:],
                                         x1[:, kc, s, tap:tap + L2],
                                         start=(ki == 0), stop=(ki == 55))
                    ki += 1
            for s in range(BS):
                tmp = vpool.tile([128, P2], F32, tag="pool2")
                nc.vector.tensor_reduce(
                    tmp[:], pss[s][:].rearrange("p (t three) -> p t three", three=3),
                    axis=AX.X, op=OP.max)
                nc.scalar.activation(x2[:, m, s, :], tmp[:], ACT.Relu,
                                     bias=bsb[2][:, m:m + 1])

        # ---- conv3..conv6: sample-batched (4 samples per matmul) ----
        def convk(xin_t, xout_t, w_ap, bias_t, Lout, ntap, pool3, out_ts=False):
            nk = 8 * ntap
            for m in range(8):
                pgs = [ppool.tile([128, 4, Lout], F32, tag="ps", name=f"psk_{m}_{gi}") for gi in range(2)]
                ki = 0
                for kc in range(8):
                    for tap in range(ntap):
                        wt = wpool.tile([128, 128], F32R, tag="w2t")
                        nc.sync.dma_start(
                            wt[:], w_ap[kc * 128:(kc + 1) * 128, tap,
                                        m * 128:(m + 1) * 128].bitcast(F32R))
                        for g in range(2):
                            nc.tensor.matmul(
                                pgs[g][:], wt[:],
                                xin_t[:, kc, 4 * g:4 * g + 4, tap:tap + Lout],
                                start=(ki == 0), stop=(ki == nk - 1))
                        ki += 1
                for g in range(2):
                    if pool3:
                        tmp = vpool.tile([128, 4, Lout // 3], F32, tag="pool6")
                        nc.vector.tensor_reduce(
                            tmp[:],
                            pgs[g][:].rearrange("p s (t three) -> p s t three",
                                                three=3),
                            axis=AX.X, op=OP.max)
                        src = tmp
                    else:
                        src = pgs[g]
                    if out_ts:
                        out_ap = xout_t[:, m, :, 4 * g:4 * g + 4].rearrange(
                            "p t s -> p s t")
                    else:
                        out_ap = xout_t[:, m, 4 * g:4 * g + 4, :]
                    nc.scalar.activation(out_ap, src[:],
                                         ACT.Relu, bias=bias_t[:, m:m + 1])

        x3 = big.tile([128, 8, BS, L3], F32R, tag="x3_x5")
        convk(x2, x3, wps[3], bsb[3], L3, 3, False)
        x4 = big.tile([128, 8, BS, L4], F32R, tag="x2_x4")
        convk(x3, x4, wps[4], bsb[4], L4, 3, False)
        x5 = big.tile([128, 8, BS, L5], F32R, tag="x3_x5")
        convk(x4, x5, wps[5], bsb[5], L5, 3, False)
        # x6 laid out [p, kc, t, s] so samples are innermost (matches feat)
        x6 = big.tile([128, 8, P6, BS], F32, tag="x6")
        convk(x5, x6, wps[6], bsb[6], L6, 3, True, out_ts=True)

        # features out: feat[g=(kc,t), j, s] = x6[j, kc, t, s]
        nc.sync.dma_start(
            feat.rearrange("g j s -> j g s"),
            x6[:].rearrange("p kc t s -> p (kc t) s"))
    nc.compile()
    return nc


def _build_fc1_nc():
    nc = bacc.Bacc("TRN2", target_bir_lowering=False, debug=False)
    f3 = nc.dram_tensor("f3", [NG, 128, B], F32, kind="ExternalInput").ap()
    fw1c = nc.dram_tensor("fw1c", [NG, 128, OCS], F32, kind="ExternalInput").ap()
    fb1c = nc.dram_tensor("fb1c", [OCS], F32, kind="ExternalInput").ap()
    h1 = nc.dram_tensor("h1", [OCS, B], F32, kind="ExternalOutput").ap()
    with tile.TileContext(nc) as tc, ExitStack() as ctx:
        big = ctx.enter_context(tc.tile_pool(name="big", bufs=1))
        wpool = ctx.enter_context(tc.tile_pool(name="wpool", bufs=6))
        vpool = ctx.enter_context(tc.tile_pool(name="vpool", bufs=2))
        ppool = ctx.enter_context(tc.tile_pool(name="ppool", bufs=4, space="PSUM"))
        feats = big.tile([128, NG, B], F32, tag="feats")
        nc.sync.dma_start(feats[:], f3.rearrange("g j s -> j g s"))
        fb1sb = big.tile([128, OCS // 128, 1], F32, tag="fb1sb")
        nc.sync.dma_start(fb1sb[:], fb1c.rearrange("(o p) -> p o", p=128)[:, :, None])
        for oc in range(OCS // 128):
            ps = ppool.tile([128, B], F32, tag="ps")
            for g in range(NG):
                wt = wpool.tile([128, 128], F32, tag="w1t")
                nc.sync.dma_start(wt[:], fw1c[g, :, oc * 128:(oc + 1) * 128])
                nc.tensor.matmul(ps[:], wt[:], feats[:, g, :],
                                 start=(g == 0), stop=(g == NG - 1))
            hsb = vpool.tile([128, B], F32, tag="hsb")
            nc.scalar.activation(hsb[:], ps[:], ACT.Relu, bias=fb1sb[:, oc])
            nc.sync.dma_start(h1[oc * 128:(oc + 1) * 128, :], hsb[:])
    nc.compile()
    return nc


def _build_fc23_nc():
    nc = bacc.Bacc("TRN2", target_bir_lowering=False, debug=False)
    h1t = nc.dram_tensor("h1t", [FC, B], F32, kind="ExternalInput").ap()
    fw2tc = nc.dram_tensor("fw2tc", [FC, OCS], F32, kind="ExternalInput").ap()
    fb2c = nc.dram_tensor("fb2c", [OCS], F32, kind="ExternalInput").ap()
    fw3tc = nc.dram_tensor("fw3tc", [OCS, NCLS], F32, kind="ExternalInput").ap()
    fb3c = nc.dram_tensor("fb3c", [NCLS], F32, kind="ExternalInput").ap()
    z3 = nc.dram_tensor("z3", [NCLS, B], F32, kind="ExternalOutput").ap()
    with tile.TileContext(nc) as tc, ExitStack() as ctx:
        big = ctx.enter_context(tc.tile_pool(name="big", bufs=1))
        wpool = ctx.enter_context(tc.tile_pool(name="wpool", bufs=4))
        ppool = ctx.enter_context(tc.tile_pool(name="ppool", bufs=4, space="PSUM"))
        h1sb = big.tile([128, FC // 128, B], F32, tag="h1sb")
        nc.sync.dma_start(h1sb[:], h1t.rearrange("(kt p) s -> p kt s", p=128))
        fb2sb = big.tile([128, OCS // 128, 1], F32, tag="fb2sb")
        nc.sync.dma_start(fb2sb[:], fb2c.rearrange("(o p) -> p o", p=128)[:, :, None])
        fb3sb = big.tile([NCLS, 1], F32, tag="fb3sb")
        nc.sync.dma_start(fb3sb[:], fb3c[:, None])
        h2sb = big.tile([128, OCS // 128, B], F32, tag="h2sb")
        for oc in range(OCS // 128):
            ps = ppool.tile([128, B], F32, tag="ps")
            for kt in range(FC // 128):
                wt = wpool.tile([128, 128], F32, tag="w2t")
                nc.sync.dma_start(wt[:], fw2tc[kt * 128:(kt + 1) * 128,
                                               oc * 128:(oc + 1) * 128])
                nc.tensor.matmul(ps[:], wt[:], h1sb[:, kt, :],
                                 start=(kt == 0), stop=(kt == FC // 128 - 1))
            nc.scalar.activation(h2sb[:, oc, :], ps[:], ACT.Relu, bias=fb2sb[:, oc])
        w3sb = big.tile([128, OCS // 128, NCLS], F32, tag="w3sb")
        nc.sync.dma_start(w3sb[:], fw3tc.rearrange("(o p) n -> p o n", p=128))
        ps3 = ppool.tile([NCLS, B], F32, tag="ps3")
        for j in range(OCS // 128):
            nc.tensor.matmul(ps3[:], w3sb[:, j, :], h2sb[:, j, :],
                             start=(j == 0), stop=(j == OCS // 128 - 1))
        z3sb = big.tile([NCLS, B], F32, tag="z3sb")
        nc.vector.tensor_scalar(z3sb[:], ps3[:], fb3sb[:], None, OP.add)
        nc.sync.dma_start(z3, z3sb[:])
    nc.compile()
    return nc


_NCS = {}


def _get_ncs():
    if not _NCS:
        _NCS["A"] = _build_conv_nc()
        _NCS["B"] = _build_fc1_nc()
        _NCS["C"] = _build_fc23_nc()
    return _NCS


def kernel(x, emb, w1, b1, w2, b2, w3, b3, w4, b4, w5, b5, w6, b6,
           fw1, fb1, fw2, fb2, fw3, fb3):
    ncs = _get_ncs()
    f32 = np.float32
    x = np.asarray(x, f32)
    # ---- host prep (layout permutes only) ----
    w1t = np.ascontiguousarray(
        np.asarray(w1, f32)[:, :E, :].transpose(2, 1, 0).reshape(224, 1024))
    wp = {2: np.ascontiguousarray(np.asarray(w2, f32).transpose(1, 2, 0))}
    for i, w in ((3, w3), (4, w4), (5, w5), (6, w6)):
        wp[i] = np.ascontiguousarray(np.asarray(w, f32).transpose(1, 2, 0))
    in_maps_a = []
    for c in CORES:
        m = {"xin": np.ascontiguousarray(
                x[c * BS:(c + 1) * BS].reshape(ROWS, C)),
             "embin": np.asarray(emb, f32),
             "w1t": w1t}
        for i in (2, 3, 4, 5, 6):
            m[f"w{i}p"] = wp[i]
        for i, b in ((1, b1), (2, b2), (3, b3), (4, b4), (5, b5), (6, b6)):
            m[f"b{i}"] = np.asarray(b, f32)
        in_maps_a.append(m)
    res_a = run_bass_kernel_spmd(ncs["A"], in_maps_a, core_ids=CORES)
    F3 = np.concatenate([res_a.results[c]["feat"] for c in CORES], axis=2)
    F3 = np.ascontiguousarray(F3)

    fw1 = np.asarray(fw1, f32)
    in_maps_b = []
    for c in CORES:
        fw1c = np.ascontiguousarray(
            fw1[c * OCS:(c + 1) * OCS].reshape(OCS, 8, 128, P6)
            .transpose(1, 3, 2, 0).reshape(NG, 128, OCS))
        in_maps_b.append({"f3": F3, "fw1c": fw1c,
                          "fb1c": np.asarray(fb1, f32)[c * OCS:(c + 1) * OCS]})
    res_b = run_bass_kernel_spmd(ncs["B"], in_maps_b, core_ids=CORES)
    h1t = np.concatenate([res_b.results[c]["h1"] for c in CORES], axis=0)
    h1t = np.ascontiguousarray(h1t)

    fw2 = np.asarray(fw2, f32)
    fw3 = np.asarray(fw3, f32)
    in_maps_c = []
    for c in CORES:
        sl = slice(c * OCS, (c + 1) * OCS)
        in_maps_c.append({
            "h1t": h1t,
            "fw2tc": np.ascontiguousarray(fw2[sl].T),
            "fb2c": np.asarray(fb2, f32)[sl],
            "fw3tc": np.ascontiguousarray(fw3[:, sl].T),
            "fb3c": (np.asarray(fb3, f32) if c == 0
                     else np.zeros(NCLS, f32)),
        })
    res_c = run_bass_kernel_spmd(ncs["C"], in_maps_c, core_ids=CORES)
    z3 = np.sum([res_c.results[c]["z3"] for c in CORES], axis=0)
    return np.ascontiguousarray(z3.T.astype(np.float32))
